# revision 1
# baseline (speedup 1.0000x reference)
"""Euclidean distance layer on 8 Trainium2 NeuronCores.

out[b, o] = || x[b, :] - weight[:, o] ||_2
x: [512, 256] f32, weight: [256, 1024] f32 -> out: [512, 1024] f32

Sharding: tensor-parallel over output features (8 x 128 columns per core).

Per core:  dist^2 = -2 * ( x~@w~_loc - 0.5*||w~_loc||^2 ) + ||x||^2
where x~, w~ are fp16 roundings of x, w (the x.w and ||w||^2 terms tolerate
fp16 easily; ||x||^2 stays fp32 -> ~2e-5 relative error on dist).
  - x~@w~ on the PE into one fp32 PSUM bank [128, 4x128]
  - ||w~||^2: fp16 squares (DVE) -> [-0.25] x2-column reduce matmul ->
    stride-0 broadcast row copy -> K=2 fp16 fold matmuls
  - ||x||^2 per-partition fp32 columns, split: batch tiles 0/1 via ACT
    Square+accum_out, tiles 2/3 via DVE mul+reduce
  - final: out = sqrt(-2 * psum + ||x||^2) on ACT (affine scale + bias)
Raw bacc, manual semaphores. Input DMAs issue in the pre-block preamble.
Host work is layout/dtype prep only.
"""

from contextlib import ExitStack

import numpy as np

B = 512      # batch
K = 256      # inputSize (contraction dim)
NOUT = 1024  # outputSize
NCORES = 8
NLOC = NOUT // NCORES  # 128 output features per core
P = 128                # partitions
KT = K // P            # 2 contraction chunks
MT = B // P            # 4 batch tiles

_NC = None  # cached compiled Bass program (same SPMD program on all cores)


def _build():
    import concourse.bass as bass
    from concourse import bacc, mybir

    f32 = mybir.dt.float32
    f16 = mybir.dt.float16
    Sqrt = mybir.ActivationFunctionType.Sqrt
    Square = mybir.ActivationFunctionType.Square
    ts = bass.ts

    nc = bacc.Bacc(
        "TRN2", target_bir_lowering=False, debug=False, num_devices=NCORES
    )

    xtf = nc.dram_tensor("xtf", [K, B], f16, kind="ExternalInput")
    xn = nc.dram_tensor("xn", [B, K], f32, kind="ExternalInput")
    wlf = nc.dram_tensor("wlf", [K, NLOC], f16, kind="ExternalInput")
    out = nc.dram_tensor("out", [B, NLOC], f32, kind="ExternalOutput")

    with ExitStack() as ctx:
        e = ctx.enter_context
        xtf_sb = e(nc.sbuf_tensor("xtfs", [P, KT, B], f16))
        wlf_sb = e(nc.sbuf_tensor("wlfs", [P, KT, NLOC], f16))
        xn_sb = [e(nc.sbuf_tensor(f"xns{h}", [P, 2, K], f32)) for h in range(2)]
        wlsq = [e(nc.sbuf_tensor(f"wlsq{k}", [P, NLOC], f16)) for k in range(KT)]
        xsq_scrA = e(nc.sbuf_tensor("xsqsA", [P, 2, K], f32))
        xsq_scrD = e(nc.sbuf_tensor("xsqsD", [P, 2, K], f32))
        xsq_colA = e(nc.sbuf_tensor("xsqcA", [P, 2], f32))
        xsq_colD = e(nc.sbuf_tensor("xsqcD", [P, 2], f32))
        neg_q = e(nc.sbuf_tensor("neg_q", [P, 2], f16))
        ones_m = e(nc.sbuf_tensor("ones_m", [2, P], f16))
        wsq_row4 = e(nc.sbuf_tensor("wsq_row4", [2, MT, NLOC], f16))
        out_sb = e(nc.sbuf_tensor("out_sb", [P, MT, NLOC], f32))
        actwarm = e(nc.sbuf_tensor("actwarm", [1, 1], f32))

        ps_w = e(nc.psum_tensor("ps_w", [2, NLOC], f32))   # -0.25*||w||^2 x2
        ps_all = e(nc.psum_tensor("ps_all", [P, MT, NLOC], f32))  # one bank

        s_wl = e(nc.semaphore("s_wl"))
        s_xt = e(nc.semaphore("s_xt"))
        s_xn = [e(nc.semaphore(f"s_xn{h}")) for h in range(2)]
        s_sq = e(nc.semaphore("s_sq"))      # 2 = both wlsq done
        s_mm = e(nc.semaphore("s_mm"))      # 1 = wsq reduce, 2+m = aug m
        s_brd = e(nc.semaphore("s_brd"))    # 1 = wsq_row4 broadcast ready
        s_colD = e(nc.semaphore("s_colD"))  # 1 = xsq cols for m2/m3 ready
        s_sqrt = e(nc.semaphore("s_sqrt"))  # m+1 = sqrt tile m in out_sb
        s_out = e(nc.semaphore("s_out"))    # 16 = sync output DMA landed
        s_out2 = e(nc.semaphore("s_out2"))  # 16 = scalar output DMA landed

        block = e(nc.Block())

        @block.sync
        def _(sync):
            sync.dma_start(
                out=xn_sb[0][:, :, :],
                in_=xn[0 : 2 * P, :].rearrange("(c p) k -> p c k", p=P),
            ).then_inc(s_xn[0], 16)
            sync.dma_start(
                out=wlf_sb[:, :, :],
                in_=wlf[:, :].rearrange("(c p) o -> p c o", p=P),
            ).then_inc(s_wl, 16)
            sync.dma_start(
                out=xtf_sb[:, :, :],
                in_=xtf[:, :].rearrange("(c p) b -> p c b", p=P),
            ).then_inc(s_xt, 16)
            sync.wait_ge(s_sqrt, 2)
            sync.dma_start(
                out=out[0 : 2 * P, :].rearrange("(m p) o -> p m o", p=P),
                in_=out_sb[:, 0:2, :],
            ).then_inc(s_out, 16)
            sync.wait_ge(s_out, 16)

        @block.gpsimd
        def _(gpsimd):
            gpsimd.dma_start(
                out=xn_sb[1][:, :, :],
                in_=xn[2 * P : 4 * P, :].rearrange("(c p) k -> p c k", p=P),
            ).then_inc(s_xn[1], 16)
            gpsimd.wait_ge(s_xn[1], 16)

        @block.scalar
        def _(scalar):
            # ||x||^2 for batch tiles 0/1 (fp32, Square + free-dim accum)
            # NOTE: also gated on s_xn[1] -- ACT accum activations racing
            # in-flight DMA traffic crash the exec unit on this stack
            scalar.wait_ge(s_xn[1], 16)
            scalar.wait_ge(s_xn[0], 16)
            for m in range(2):
                scalar.activation(
                    xsq_scrA[:, m, :], xn_sb[0][:, m, :], Square,
                    accum_out=xsq_colA[:, m : m + 1],
                )
            scalar.drain()  # ACT RAW: sqrts below read xsq_colA
            for m in range(MT):
                scalar.wait_ge(s_mm, 2 + m)
                if m == 2:
                    scalar.wait_ge(s_colD, 1)
                bias = (
                    xsq_colA[:, m : m + 1] if m < 2
                    else xsq_colD[:, m - 2 : m - 1]
                )
                scalar.activation(
                    out_sb[:, m, :], ps_all[:, m, :], Sqrt,
                    bias=bias, scale=-2.0,
                ).then_inc(s_sqrt)
            scalar.wait_ge(s_sqrt, MT)
            scalar.dma_start(
                out=out[2 * P : 4 * P, :].rearrange("(m p) o -> p m o", p=P),
                in_=out_sb[:, 2:4, :],
            ).then_inc(s_out2, 16)
            scalar.wait_ge(s_out2, 16)


        @block.vector
        def _(vector):
            vector.memset(neg_q[:, :], -0.25)
            vector.memset(ones_m[:, :], 1.0)
            vector.wait_ge(s_wl, 16)
            vector.tensor_mul(wlsq[0][:, :], wlf_sb[:, 0, :], wlf_sb[:, 0, :])
            vector.tensor_mul(
                wlsq[1][:, :], wlf_sb[:, 1, :], wlf_sb[:, 1, :]
            ).then_inc(s_sq, 2)
            # ||x||^2 for batch tiles 2/3: fp32 squares
            vector.wait_ge(s_xn[1], 16)
            for m in range(2):
                vector.tensor_mul(
                    xsq_scrD[:, m, :], xn_sb[1][:, m, :], xn_sb[1][:, m, :]
                )
            # broadcast -0.25*||w||^2 rows across the 4 m-slices
            vector.wait_ge(s_mm, 1)
            vector.tensor_copy(
                wsq_row4[:, :, :],
                bass.AP(tensor=ps_w, offset=0, ap=[[NLOC, 2], [0, MT], [1, NLOC]]),
            ).then_inc(s_brd)
            vector.drain()  # DVE RAW: reduces read xsq_scrD
            for m in range(2):
                inst = vector.tensor_reduce(
                    xsq_colD[:, m : m + 1], xsq_scrD[:, m, :],
                    axis=mybir.AxisListType.X, op=mybir.AluOpType.add,
                )
            inst.then_inc(s_colD)

        @block.tensor
        def _(tensor):
            # -0.25*||w||^2 reduce, two identical rows
            tensor.wait_ge(s_sq, 2)
            tensor.matmul(
                ps_w[:, :], lhsT=neg_q[:, :], rhs=wlsq[0][:, :],
                start=True, stop=False,
            )
            tensor.matmul(
                ps_w[:, :], lhsT=neg_q[:, :], rhs=wlsq[1][:, :],
                start=False, stop=True,
            ).then_inc(s_mm)  # = 1
            # main fp16 matmuls: one PSUM bank, single start on the first
            tensor.wait_ge(s_xt, 16)
            for k in range(KT):
                for m in range(MT):
                    tensor.matmul(
                        ps_all[:, m, :],
                        lhsT=xtf_sb[:, k, ts(m, P)],
                        rhs=wlf_sb[:, k, :],
                        start=(k == 0 and m == 0), stop=False,
                        skip_group_check=True,
                    )
            # fold -0.5*||w||^2 per m-slice (K=2: two -0.25 rows)
            tensor.wait_ge(s_brd, 1)
            for m in range(MT):
                tensor.matmul(
                    ps_all[:, m, :],
                    lhsT=ones_m[:, :],
                    rhs=wsq_row4[:, m, :],
                    start=False, stop=True, skip_group_check=True,
                ).then_inc(s_mm)  # = 2 + m

    nc.compile()
    return nc


def _get_nc():
    global _NC
    if _NC is None:
        _NC = _build()
    return _NC


def _make_in_maps(x: np.ndarray, weight: np.ndarray):
    x = np.ascontiguousarray(x.astype(np.float32, copy=False))
    xtf = np.ascontiguousarray(x.T.astype(np.float16))
    wf = weight.astype(np.float16)
    return [
        {
            "xtf": xtf,
            "xn": x,
            "wlf": np.ascontiguousarray(wf[:, c * NLOC : (c + 1) * NLOC]),
        }
        for c in range(NCORES)
    ]


def run(x: np.ndarray, weight: np.ndarray, trace: bool = False):
    """Returns (full_output, BassKernelResults)."""
    from concourse.bass_utils import run_bass_kernel_spmd

    nc = _get_nc()
    res = run_bass_kernel_spmd(
        nc, _make_in_maps(x, weight), core_ids=list(range(NCORES)), trace=trace
    )
    full = np.concatenate(
        [res.results[c]["out"] for c in range(NCORES)], axis=1
    )
    return full, res


def kernel(x: np.ndarray, weight: np.ndarray) -> np.ndarray:
    return run(x, weight)[0]



# revision 10
# speedup vs baseline: 1.1734x; 1.1734x over previous
"""Euclidean distance layer on 8 Trainium2 NeuronCores — v2.

out[b, o] = || x[b, :] - weight[:, o] ||_2
x: [512, 256] f32, weight: [256, 1024] f32 -> out: [512, 1024] f32

Sharding: tensor-parallel over output features (8 x 128 columns per core).

Per core, with xt := -x/2 shipped fp16 (exact power-of-2 rescale):
  psum[o, b]  = sum_k w[k,o] xt[b,k]           = -0.5 x.w     (PE)
  psum[o, b] += sum_k xt[b,k]^2 (all-ones lhsT) = +0.25||x||^2 (PE)
  out[o, b]   = sqrt(4 psum + ||w_o||^2)                      (ACT)
  ||w_o||^2 column: PE ones-reduce over DVE squares of w -> ACT bias.

Output is fp16 in [o, b] layout; the host transposes/casts to f32.

Structural tricks (why this beats the v1 kernel):
  - NO nc.Block(): the NEFF epilogue makes each engine serially clear ~51
    hardware semaphores (Tensor: ~7us!). Without the block-end barrier each
    engine falls into its clear sequence as soon as its OWN stream ends, so
    Tensor's 7us runs concurrently with the sqrt/output-DMA tail instead of
    after it.
  - Semaphore padding: all live semaphores are pushed into [207, 255] --
    the range cleared by the Sync engine, which is the last to retire --
    so no other engine's early clears can kill a live semaphore.
  - Sqrt activation-table prewarm as ACT's first instruction (1.28us table
    load runs at t=0 under the input DMAs, not before the first real sqrt).
  - The all-ones [128,128] lhsT matmul broadcast-accumulates the ||x||^2
    row into every PSUM partition -- no PSUM->SBUF->PE round trip.
  - fp16-only inputs (320KB vs v1's 832KB), fp16 output, one DMA per
    tensor, issued from two engines in parallel at t=0.
"""

from contextlib import ExitStack

import numpy as np

B = 512      # batch
K = 256      # inputSize (contraction dim)
NOUT = 1024  # outputSize
NCORES = 8
NLOC = NOUT // NCORES  # 128 output features per core
P = 128                # partitions
KT = K // P            # 2 contraction chunks
HB = B // 2            # 256-batch halves for sqrt/output pipelining

_NC = None  # cached compiled Bass program (same SPMD program on all cores)


def _build():
    import concourse.bass as bass
    from concourse import bacc, mybir

    f32 = mybir.dt.float32
    f16 = mybir.dt.float16
    Sqrt = mybir.ActivationFunctionType.Sqrt
    Copy = mybir.ActivationFunctionType.Copy

    nc = bacc.Bacc(
        "TRN2", target_bir_lowering=False, debug=False, num_devices=NCORES
    )

    xt0 = nc.dram_tensor("xt0", [P, B], f16, kind="ExternalInput")
    xt1 = nc.dram_tensor("xt1", [P, B], f16, kind="ExternalInput")
    wl = nc.dram_tensor("wl", [P, KT, NLOC], f16, kind="ExternalInput")
    out = nc.dram_tensor("out", [P, B], f16, kind="ExternalOutput")

    with ExitStack() as ctx:
        e = ctx.enter_context

        # --- semaphore layout control -------------------------------------
        # The NEFF epilogue clears sem ranges per engine:
        #   PE: 2-53, ACT: 54-104, Pool: 105-155, DVE: 156-206, SP: 207-255.
        # SP (sync) retires last (it issues the final output DMAs), so every
        # semaphore that can still be incremented or waited on late in the
        # program must live in [207, 255]. Pad allocations up to 206.
        pad = nc.alloc_semaphore("pad0")
        assert pad.num <= 206, pad.num
        i = 1
        while True:
            p_ = nc.alloc_semaphore(f"pad{i}")
            i += 1
            if p_.num >= 206:
                break
        s_w = nc.alloc_semaphore("s_w")
        assert s_w.num == 207, s_w.num
        s_x0 = nc.alloc_semaphore("s_x0")
        s_x1 = nc.alloc_semaphore("s_x1")
        s_wsq = nc.alloc_semaphore("s_wsq")
        s_xsqs = nc.alloc_semaphore("s_xsqs")
        s_wcolp = nc.alloc_semaphore("s_wcolp")
        s_fold = [nc.alloc_semaphore(f"s_fold{h}") for h in range(2)]
        s_sq = [nc.alloc_semaphore(f"s_sq{h}") for h in range(2)]
        s_out = nc.alloc_semaphore("s_out")

        # --- on-chip tensors ----------------------------------------------
        xt_sb = e(nc.sbuf_tensor("xt_sb", [P, KT, B], f16))
        wl_sb = e(nc.sbuf_tensor("wl_sb", [P, KT, NLOC], f16))
        wsq_sb = e(nc.sbuf_tensor("wsq_sb", [P, KT, NLOC], f16))
        xsq_sb = e(nc.sbuf_tensor("xsq_sb", [P, KT, B], f16))
        xsqs_sb = e(nc.sbuf_tensor("xsqs_sb", [P, B], f16))
        ones_sb = e(nc.sbuf_tensor("ones_sb", [P, P], f16))
        wcol_sb = e(nc.sbuf_tensor("wcol_sb", [P, 1], f32))
        out_sb = e(nc.sbuf_tensor("out_sb", [P, B], f16))
        warm_sb = e(nc.sbuf_tensor("warm_sb", [1, 1], f32))

        ps_dist = e(nc.psum_tensor("ps_dist", [P, B], f32))
        ps_wcol = e(nc.psum_tensor("ps_wcol", [P, 1], f32))

        # --- engine streams (no Block, no end barrier) --------------------
        # sync: input DMAs for x at t0; output DMAs at the end.
        nc.sync.dma_start(out=xt_sb[:, 0, :], in_=xt0[:, :]).then_inc(s_x0, 16)
        nc.sync.dma_start(out=xt_sb[:, 1, :], in_=xt1[:, :]).then_inc(s_x1, 16)

        # gpsimd: weight DMA at t0, then retires (its sem-clear range holds
        # only padding and the framework entry-barrier sems).
        nc.gpsimd.dma_start(out=wl_sb[:, :, :], in_=wl[:, :, :]).then_inc(s_w, 16)

        # vector (DVE): the elementwise squares. Retires early.
        nc.vector.memset(ones_sb[:, :], 1.0)
        nc.vector.wait_ge(s_w, 16)
        nc.vector.tensor_mul(
            wsq_sb[:, :, :], wl_sb[:, :, :], wl_sb[:, :, :]
        ).then_inc(s_wsq)
        nc.vector.wait_ge(s_x0, 16)
        nc.vector.tensor_mul(xsq_sb[:, 0, :], xt_sb[:, 0, :], xt_sb[:, 0, :])
        nc.vector.wait_ge(s_x1, 16)
        nc.vector.tensor_mul(xsq_sb[:, 1, :], xt_sb[:, 1, :], xt_sb[:, 1, :])
        nc.vector.tensor_add(
            xsqs_sb[:, :], xsq_sb[:, 0, :], xsq_sb[:, 1, :]
        ).then_inc(s_xsqs)

        # tensor (PE): ||w||^2 column, main matmuls, ||x||^2 broadcast fold.
        # Retires at ~4.6us so its 7us sem-clear epilogue overlaps the tail.
        nc.tensor.wait_ge(s_wsq, 1)
        nc.tensor.matmul(
            ps_wcol[:, :], lhsT=wsq_sb[:, 0, :], rhs=ones_sb[:, 0:1],
            start=True, stop=False,
        )
        nc.tensor.matmul(
            ps_wcol[:, :], lhsT=wsq_sb[:, 1, :], rhs=ones_sb[:, 0:1],
            start=False, stop=True, skip_group_check=True,
        ).then_inc(s_wcolp)
        nc.tensor.wait_ge(s_x0, 16)
        nc.tensor.matmul(
            ps_dist[:, :], lhsT=wl_sb[:, 0, :], rhs=xt_sb[:, 0, :],
            start=True, stop=False,
        )
        nc.tensor.wait_ge(s_x1, 16)
        nc.tensor.matmul(
            ps_dist[:, :], lhsT=wl_sb[:, 1, :], rhs=xt_sb[:, 1, :],
            start=False, stop=False, skip_group_check=True,
        )
        nc.tensor.wait_ge(s_xsqs, 1)
        for h in range(2):
            nc.tensor.matmul(
                ps_dist[:, h * HB : (h + 1) * HB],
                lhsT=ones_sb[:, :],
                rhs=xsqs_sb[:, h * HB : (h + 1) * HB],
                start=False, stop=True, skip_group_check=True,
            ).then_inc(s_fold[h])

        # scalar (ACT): ||w||^2 bias column copy, then the two half-batch
        # sqrts: out = sqrt(4*psum + ||w||^2). ACT activations (and the
        # activation-table load) crash this stack when they race in-flight
        # DMA data, so ACT's first op is gated on ALL input DMAs; the
        # compiler places the sqrt-table load right after these waits,
        # where it overlaps the PE matmuls.
        nc.scalar.wait_ge(s_w, 16)
        nc.scalar.wait_ge(s_x0, 16)
        nc.scalar.wait_ge(s_x1, 16)
        nc.scalar.wait_ge(s_wcolp, 1)
        nc.scalar.activation(wcol_sb[:, :], ps_wcol[:, :], Copy)
        for h in range(2):
            nc.scalar.wait_ge(s_fold[h], 1)
            nc.scalar.activation(
                out_sb[:, h * HB : (h + 1) * HB],
                ps_dist[:, h * HB : (h + 1) * HB],
                Sqrt, bias=wcol_sb[:, :], scale=4.0,
            ).then_inc(s_sq[h])

        # sync again: output DMAs. SP waits for their completion so it is
        # the last engine to retire and its sem-clear range stays clean.
        for h in range(2):
            nc.sync.wait_ge(s_sq[h], 1)
            nc.sync.dma_start(
                out=out[:, h * HB : (h + 1) * HB],
                in_=out_sb[:, h * HB : (h + 1) * HB],
            ).then_inc(s_out, 16)
        nc.sync.wait_ge(s_out, 32)

    nc.compile()
    return nc


def _get_nc():
    global _NC
    if _NC is None:
        _NC = _build()
    return _NC


def _make_in_maps(x: np.ndarray, weight: np.ndarray):
    x = np.asarray(x, dtype=np.float32)
    weight = np.asarray(weight, dtype=np.float32)
    # xt = -x/2: exact exponent shift; makes psum = -0.5 x.w + 0.25||x||^2
    xt = np.ascontiguousarray((x.T * -0.5).astype(np.float16))  # [K, B]
    xt0 = np.ascontiguousarray(xt[0:P])
    xt1 = np.ascontiguousarray(xt[P : 2 * P])
    w16 = weight.astype(np.float16)
    maps = []
    for c in range(NCORES):
        wc = w16[:, c * NLOC : (c + 1) * NLOC]             # [K, NLOC]
        wlc = np.ascontiguousarray(
            wc.reshape(KT, P, NLOC).transpose(1, 0, 2)     # [P, KT, NLOC]
        )
        maps.append({"xt0": xt0, "xt1": xt1, "wl": wlc})
    return maps


def run(x: np.ndarray, weight: np.ndarray, trace: bool = False):
    """Returns (full_output, BassKernelResults)."""
    from concourse.bass_utils import run_bass_kernel_spmd

    nc = _get_nc()
    res = run_bass_kernel_spmd(
        nc, _make_in_maps(x, weight), core_ids=list(range(NCORES)), trace=trace
    )
    full = np.concatenate(
        [res.results[c]["out"].T for c in range(NCORES)], axis=1
    ).astype(np.float32)
    return full, res


def kernel(x: np.ndarray, weight: np.ndarray) -> np.ndarray:
    return run(x, weight)[0]
